# revision 1
# baseline (speedup 1.0000x reference)
"""Trainium2 Bass kernel for nn_DataEmbedding_ALLPE_Weighted.

Sharding: sequence-parallel over L (4096/8 = 512 positions per core, all 16
batches). Position-dependent tensors (tape_pos, learned_pe, fixed-PE fold)
shard 1/8 per core with no replication and no collectives; the rolling
window needs a 23-step left halo of x (host-baked), and the circular conv
needs one feature column per side (host-computed, DMA'd into comb).

Per-core schedule:
  consts -> row broadcasts (PE rank-1) -> B (pe_lin) -> C (R0) ->
  A (rolling stats, 4-batch tiles, partition = ch*4+bl) ->
  D: software-pipelined over 16 batches; per batch: LN-c for batch b
     (bn_stats + smalls batched over the 4 token tiles), fused DMA-crossbar
     transposes, conv MMs of batch b+1 interleaved with mixer MMs of batch
     b on PE, LN-t, weighted-sum epilogue (Pool o1/o2, DVE m1/osb), one
     fused out-DMA per batch.
"""

import numpy as np

import concourse.bass as bass
import concourse.mybir as mybir
import concourse.tile as tile
from concourse import bacc
from concourse.bass_utils import run_bass_kernel_spmd

F32 = mybir.dt.float32
BF16 = mybir.dt.bfloat16
AL = mybir.AluOpType
AF = mybir.ActivationFunctionType

B, L, C, D = 16, 4096, 32, 512
W = 24
LAGS = [3, 5, 7]
EPS = 1e-5
NCORES = 8
LC = L // NCORES           # 512 positions per core
NBT = 4                    # batch tiles of 4 batches
NTL = LC // 128            # 4 token tiles per batch
PADL = W - 1               # 23
XSL = LC + PADL            # 535
CB = LC + 2                # 514 comb cols per batch


def _build_bass():
    nc = bacc.Bacc("TRN2", target_bir_lowering=False, debug=False,
                   num_devices=NCORES)

    def din(name, shape, dt=F32):
        return nc.dram_tensor(name, shape, dt, kind="ExternalInput").ap()

    t_xs = din("xs", [NBT, 128, XSL])
    t_halo = din("halo", [2, 128, 2 * B], BF16)
    t_wr = din("wr", [6, 128, 512], BF16)        # idx = tap*2 + half
    t_convb = din("convb", [1, 512], BF16)
    t_ftg = din("ftg", [4, 128, 512], BF16)
    t_m2t = din("m2t", [4, 128, 512], BF16)
    t_c1 = din("c1", [1, 512], BF16)
    t_tapetc = din("tapetc", [4, 128, 512], BF16)
    t_pelpf = din("pelpf", [NTL, 128, 1024])
    t_grows = din("grows", [1, 3 * 512])         # w0*g_c | w3*g_t | w2*g_l
    t_ident = din("ident", [128, 128], BF16)
    t_ones1 = din("ones1", [1, 128], BF16)
    t_ones1f = din("ones1f", [1, 128])
    t_out = nc.dram_tensor("out", [B, LC, D], F32, kind="ExternalOutput").ap()

    with tile.TileContext(nc) as tc:
        _body(tc, nc, t_xs, t_halo, t_wr, t_convb, t_ftg, t_m2t, t_c1,
              t_tapetc, t_pelpf, t_grows, t_ident, t_ones1, t_ones1f, t_out)
    nc.compile()
    return nc


def _body(tc, nc, t_xs, t_halo, t_wr, t_convb, t_ftg, t_m2t, t_c1,
          t_tapetc, t_pelpf, t_grows, t_ident, t_ones1, t_ones1f, t_out):
    v = nc.vector
    gp = nc.gpsimd
    sc = nc.scalar
    pe = nc.tensor
    sy = nc.sync

    cpool = tc.alloc_tile_pool(name="consts", bufs=1)
    wr_s = cpool.tile([128, 6 * 512], BF16)
    sy.dma_start(wr_s[:].rearrange("p (k n) -> p k n", k=6),
                 t_wr.rearrange("k p n -> p k n"))
    ftg_s = cpool.tile([128, 4 * 512], BF16)
    sy.dma_start(ftg_s[:].rearrange("p (k n) -> p k n", k=4),
                 t_ftg.rearrange("k p n -> p k n"))
    ident_s = cpool.tile([128, 128], BF16)
    sy.dma_start(ident_s[:], t_ident)
    ones1_s = cpool.tile([1, 128], BF16)
    sy.dma_start(ones1_s[:], t_ones1)
    ones1f_s = cpool.tile([1, 128], F32)
    sy.dma_start(ones1f_s[:], t_ones1f)
    convb_s = cpool.tile([1, 512], BF16)
    sy.dma_start(convb_s[:], t_convb)
    c1_s = cpool.tile([1, 512], BF16)
    sy.dma_start(c1_s[:], t_c1)
    grows_s = cpool.tile([1, 3 * 512], F32)
    sy.dma_start(grows_s[:], t_grows)
    eps_s = cpool.tile([128, 1], F32)
    gp.memset(eps_s[:], EPS)
    zscan_s = cpool.tile([128, XSL], F32)
    gp.memset(zscan_s[:], 0.0)

    # SBUF residents
    bc_s = cpool.tile([128, 3 * 512], F32)       # g0bc | g3bc | glbc
    bcb_s = cpool.tile([128, 2 * 512], BF16)     # bf16 g0bc | g3bc
    pelin_sb = cpool.tile([128, NTL * 512], BF16)
    r0_sb = cpool.tile([128, NTL * 512], BF16)
    comb = [[cpool.tile([128, 4 * CB], BF16, name=f"comb{_t}_{_h}")
             for _h in range(2)] for _t in range(NBT)]

    psA = tc.alloc_tile_pool(name="psA", bufs=4, space="PSUM")
    psB = tc.alloc_tile_pool(name="psB", bufs=4, space="PSUM")
    pa = tc.alloc_tile_pool(name="pa", bufs=2)
    pm = tc.alloc_tile_pool(name="pm", bufs=2)
    st = tc.alloc_tile_pool(name="st", bufs=4)
    pst = tc.alloc_tile_pool(name="pst", bufs=1)

    # ---- row broadcasts via rank-1 f32 matmuls ----
    for i in range(3):
        ps = psA.tile([128, 512], F32, tag="ph")
        pe.matmul(ps[:], ones1f_s[:], grows_s[:, 512 * i:512 * (i + 1)],
                  start=True, stop=True)
        sc.copy(bc_s[:, 512 * i:512 * (i + 1)], ps[:])
    sc.copy(bcb_s[:], bc_s[:, 0:1024])
    g0bf = bcb_s[:, 0:512]
    g3bf = bcb_s[:, 512:1024]
    glbc = bc_s[:, 1024:1536]

    # ---- halo columns into comb (one DMA per (t, half)) ----
    for t in range(NBT):
        for h in range(2):
            dstv = comb[t][h][:, :].rearrange("p (b k) -> p b k", b=4)
            srcv = t_halo[h, :, 8 * t:8 * (t + 1)].rearrange(
                "p (b s) -> p b s", s=2)
            sy.dma_start(dstv[:, :, 0:1], srcv[:, :, 0:1])
            sy.dma_start(dstv[:, :, CB - 1:CB], srcv[:, :, 1:2])

    # ---- Phase B: pe_lin ----
    tg = pst.tile([128, 4 * 512], BF16, tag="tg")
    sy.dma_start(tg[:].rearrange("p (k n) -> p k n", k=4),
                 t_tapetc.rearrange("k p n -> p k n"))
    m2 = pst.tile([128, 4 * 512], BF16, tag="m2")
    sy.dma_start(m2[:].rearrange("p (k n) -> p k n", k=4),
                 t_m2t.rearrange("k p n -> p k n"))
    for il in range(NTL):
        ps = psB.tile([128, 512], F32, tag="ppt")
        for j in range(4):
            pe.matmul(ps[:], tg[:, 512 * j + 128 * il:512 * j + 128 * (il + 1)],
                      m2[:, 512 * j:512 * (j + 1)],
                      start=(j == 0), stop=False)
        pe.matmul(ps[:], ones1_s[:], c1_s[:], start=False, stop=True)
        sc.copy(pelin_sb[:, 512 * il:512 * (il + 1)], ps[:])

    # ---- Phase C: R0 ----
    plf = pst.tile([128, NTL * 1024], F32, tag="plf")
    sy.dma_start(plf[:].rearrange("p (k n) -> p k n", k=NTL),
                 t_pelpf.rearrange("k p n -> p k n"))
    for il in range(NTL):
        pl = plf[:, 1024 * il:1024 * il + 512]
        pf = plf[:, 1024 * il + 512:1024 * (il + 1)]
        bn = st.tile([128, 6], F32, tag="bnC")
        v.bn_stats(bn[:], pl)
        mv = st.tile([128, 2], F32, tag="mvC")
        v.bn_aggr(mv[:], bn[:])
        se = st.tile([128, 1], F32, tag="seC")
        sc.activation(se[:], mv[:, 1:2], AF.Sqrt, bias=eps_s[:])
        al = st.tile([128, 1], F32, tag="alC")
        v.reciprocal(al[:], se[:])
        be = st.tile([128, 1], F32, tag="beC")
        v.scalar_tensor_tensor(be[:], mv[:, 0:1], -1.0, al[:],
                               op0=AL.mult, op1=AL.mult)
        hl = pm.tile([128, 512], BF16, tag="hl")
        sc.activation(hl[:], pl, AF.Identity, bias=be[:], scale=al[:])
        t1 = pm.tile([128, 512], F32, tag="t1")
        gp.tensor_tensor(t1[:], hl[:], glbc, op=AL.mult)
        gp.tensor_tensor(r0_sb[:, 512 * il:512 * (il + 1)], t1[:], pf,
                         op=AL.add)

    # ---- Phase A: rolling stats -> comb ----
    for t in range(NBT):
        xs = pa.tile([128, XSL], F32, tag="xs")
        sy.dma_start(xs[:], t_xs[t])
        xsb = pa.tile([128, XSL], BF16, tag="xsb")
        v.tensor_copy(xsb[:], xs[:])
        xsq = pa.tile([128, XSL], F32, tag="xsq")
        sc.activation(xsq[:], xs[:], AF.Square)
        cs = pa.tile([128, XSL + 1], F32, tag="cs")
        gp.memset(cs[:, 0:1], 0.0)
        v.tensor_tensor_scan(cs[:, 1:XSL + 1], zscan_s[:], xs[:], 0.0,
                             op0=AL.add, op1=AL.add)
        cs2 = pa.tile([128, XSL + 1], F32, tag="cs2")
        gp.memset(cs2[:, 0:1], 0.0)
        v.tensor_tensor_scan(cs2[:, 1:XSL + 1], zscan_s[:], xsq[:], 0.0,
                             op0=AL.add, op1=AL.add)
        wsum = pa.tile([128, LC], BF16, tag="wsum")
        v.tensor_sub(wsum[:], cs[:, W:XSL + 1], cs[:, 0:LC])
        ssqw = pa.tile([128, LC], F32, tag="ssqw")
        gp.tensor_sub(ssqw[:], cs2[:, W:XSL + 1], cs2[:, 0:LC])
        wsq = pa.tile([128, LC], F32, tag="wsq")
        v.tensor_mul(wsq[:], wsum[:], wsum[:])
        var23 = pa.tile([128, LC], F32, tag="var23")
        v.scalar_tensor_tensor(var23[:], wsq[:], -1.0 / W, ssqw[:],
                               op0=AL.mult, op1=AL.add)
        v.tensor_scalar_max(var23[:], var23[:], 0.0)
        stdt = pa.tile([128, LC], BF16, tag="stdt")
        sc.activation(stdt[:], var23[:], AF.Sqrt, scale=1.0 / (W - 1))
        # bf16 log-shift chains, both on DVE (2x mode)
        outs = {}
        for name, op in (("mx", AL.max), ("mn", AL.min)):
            m2c = pa.tile([128, XSL - 1], BF16, tag=name + "2")
            v.tensor_tensor(m2c[:], xsb[:, 0:XSL - 1], xsb[:, 1:XSL], op=op)
            m4 = pa.tile([128, XSL - 3], BF16, tag=name + "4")
            v.tensor_tensor(m4[:], m2c[:, 0:XSL - 3], m2c[:, 2:XSL - 1], op=op)
            m8 = pa.tile([128, XSL - 7], BF16, tag=name + "8")
            v.tensor_tensor(m8[:], m4[:, 0:XSL - 7], m4[:, 4:XSL - 3], op=op)
            m16 = pa.tile([128, XSL - 15], BF16, tag=name + "16")
            v.tensor_tensor(m16[:], m8[:, 0:XSL - 15], m8[:, 8:XSL - 7], op=op)
            mo = pa.tile([128, LC], BF16, tag=name + "o")
            v.tensor_tensor(mo[:], m16[:, 8:LC + 8], m8[:, 0:LC], op=op)
            outs[name] = mo
        lags = []
        for lg_ in LAGS:
            lg = pa.tile([128, LC], BF16, tag=f"lag{lg_}")
            v.tensor_sub(lg[:], xsb[:, PADL:XSL], xsb[:, PADL - lg_:XSL - lg_])
            lags.append(lg)

        feats = [[xsb[:, PADL:XSL], wsum[:], outs["mx"][:], outs["mn"][:]],
                 [stdt[:]] + [lg[:] for lg in lags]]
        for h in range(2):
            for r, ft in enumerate(feats[h]):
                dst = comb[t][h][32 * r:32 * (r + 1), :].rearrange(
                    "p (b k) -> p b k", k=CB)
                sy.dma_start(dst[:, :, 1:1 + LC], ft)

    # ---- Phase D: software-pipelined over batches ----
    def conv_tiles(b):
        t, bl = b // 4, b % 4
        phs = []
        for il in range(NTL):
            col = CB * bl + 128 * il
            ph = psA.tile([128, 512], F32, tag="ph")
            k = 0
            for tap in range(3):
                for h in range(2):
                    pe.matmul(ph[:], comb[t][h][:, col + tap:col + tap + 128],
                              wr_s[:, 512 * (tap * 2 + h):
                                   512 * (tap * 2 + h) + 512],
                              start=(k == 0), stop=False)
                    k += 1
            pe.matmul(ph[:], ones1_s[:], convb_s[:], start=False, stop=True)
            phs.append(ph)
        return phs

    def ln_smalls(srcs, tagp):
        mvg = st.tile([128, 8], F32, tag="mv" + tagp)
        mvv = mvg[:].rearrange("p (a b) -> p a b", a=2)
        for il in range(NTL):
            bn = st.tile([128, 6], F32, tag="bn")
            v.bn_stats(bn[:], srcs[il][:])
            v.bn_aggr(mvv[:, :, il:il + 1], bn[:])
        se4 = st.tile([128, 4], F32, tag="se" + tagp)
        sc.activation(se4[:], mvg[:, 4:8], AF.Sqrt, bias=eps_s[:])
        al4 = st.tile([128, 4], F32, tag="al" + tagp)
        v.reciprocal(al4[:], se4[:])
        be4 = st.tile([128, 4], F32, tag="be" + tagp)
        v.scalar_tensor_tensor(be4[:], mvg[:, 0:4], -1.0, al4[:],
                               op0=AL.mult, op1=AL.mult)
        return al4, be4

    phs = conv_tiles(0)
    for b in range(B):
        # LN-c
        al4, be4 = ln_smalls(phs, "c")
        hcs = []
        hcTs = []
        for il in range(NTL):
            hc = pm.tile([128, 512], BF16, tag="hc")
            sc.activation(hc[:], phs[il][:], AF.Identity,
                          bias=be4[:, il:il + 1], scale=al4[:, il:il + 1])
            hcs.append(hc)
        for il in range(NTL):
            hcT = pm.tile([128, 512], BF16, tag="hcT")
            sc.dma_start_transpose(
                hcT[:].rearrange("p (j n) -> p j n", j=4), hcs[il][:])
            hcTs.append(hcT)
        # PE: next batch's convs interleaved with this batch's mixer MMs
        ppts = []
        nphs = []
        for il in range(NTL):
            if b + 1 < B:
                nphs_il = None
                t2, bl2 = (b + 1) // 4, (b + 1) % 4
                col = CB * bl2 + 128 * il
                ph2 = psA.tile([128, 512], F32, tag="ph")
                k = 0
                for tap in range(3):
                    for h in range(2):
                        pe.matmul(ph2[:],
                                  comb[t2][h][:, col + tap:col + tap + 128],
                                  wr_s[:, 512 * (tap * 2 + h):
                                       512 * (tap * 2 + h) + 512],
                                  start=(k == 0), stop=False)
                        k += 1
                pe.matmul(ph2[:], ones1_s[:], convb_s[:],
                          start=False, stop=True)
                nphs.append(ph2)
            ppt = psB.tile([128, 512], F32, tag="ppt")
            for j in range(4):
                pe.matmul(ppt[:], hcTs[il][:, 128 * j:128 * (j + 1)],
                          ftg_s[:, 512 * j:512 * (j + 1)],
                          start=(j == 0), stop=False)
            pe.matmul(ppt[:], ident_s[:],
                      pelin_sb[:, 512 * il:512 * (il + 1)],
                      start=False, stop=True)
            ppts.append(ppt)
        # LN-t
        al4t, be4t = ln_smalls(ppts, "t")
        hts = []
        for il in range(NTL):
            ht = pm.tile([128, 512], BF16, tag="ht")
            sc.activation(ht[:], ppts[il][:], AF.Identity,
                          bias=be4t[:, il:il + 1], scale=al4t[:, il:il + 1])
            hts.append(ht)
        # epilogue
        osb4 = pm.tile([128, NTL * 512], F32, tag="osb4")
        for il in range(NTL):
            o1 = pm.tile([128, 512], BF16, tag="o1")
            gp.tensor_tensor(o1[:], hcs[il][:], g0bf, op=AL.mult)
            o2 = pm.tile([128, 512], BF16, tag="o2")
            gp.tensor_tensor(o2[:], o1[:], r0_sb[:, 512 * il:512 * (il + 1)],
                             op=AL.add)
            m1 = pm.tile([128, 512], BF16, tag="m1")
            v.tensor_tensor(m1[:], hts[il][:], g3bf, op=AL.mult)
            v.tensor_tensor(osb4[:, 512 * il:512 * (il + 1)], m1[:], o2[:],
                            op=AL.add)
        sy.dma_start(t_out[b].rearrange("(i p) d -> p i d", p=128),
                     osb4[:].rearrange("p (i d) -> p i d", i=NTL))
        phs = nphs

    for p in (pst, st, pm, pa, psB, psA, cpool):
        p.release()


_NC_CACHE = None


def _get_nc():
    global _NC_CACHE
    if _NC_CACHE is None:
        _NC_CACHE = _build_bass()
    return _NC_CACHE


def _to_bf16(a):
    import ml_dtypes
    return np.asarray(a, np.float32).astype(ml_dtypes.bfloat16)


def _fixed_pe_fold():
    """Normalized fixed sinusoidal PE table [L, D] (input-independent)."""
    pos = np.arange(L, dtype=np.float64)
    div = np.exp(np.arange(0, D, 2, dtype=np.float64) * (-np.log(10000.0) / D))
    ang = pos[:, None] * div[None, :]
    tab = np.zeros((L, D), np.float64)
    tab[:, 0::2] = np.sin(ang)
    tab[:, 1::2] = np.cos(ang)
    tab = tab.astype(np.float32)
    m = tab.mean(-1, keepdims=True)
    vv = ((tab - m) ** 2).mean(-1, keepdims=True)
    return (tab - m) / np.sqrt(vv + EPS)


_TABN = _fixed_pe_fold()


def _host_prep(inputs):
    f32 = np.float32
    x = np.asarray(inputs["x"], f32)
    conv_w = np.asarray(inputs["conv_w"], f32)
    conv_b = np.asarray(inputs["conv_b"], f32)
    learned_pe = np.asarray(inputs["learned_pe"], f32)
    tape_pos = np.asarray(inputs["tape_pos"], f32)
    tproj_w = np.asarray(inputs["tproj_w"], f32)
    tproj_b = np.asarray(inputs["tproj_b"], f32)
    mixer_w = np.asarray(inputs["mixer_w"], f32)
    mixer_b = np.asarray(inputs["mixer_b"], f32)
    g_c, b_c = np.asarray(inputs["g_c"], f32), np.asarray(inputs["b_c"], f32)
    g_f, b_f = np.asarray(inputs["g_f"], f32), np.asarray(inputs["b_f"], f32)
    g_l, b_l = np.asarray(inputs["g_l"], f32), np.asarray(inputs["b_l"], f32)
    g_t, b_t = np.asarray(inputs["g_t"], f32), np.asarray(inputs["b_t"], f32)
    wp = np.asarray(inputs["weight_params"], f32)

    e = np.exp(wp - wp.max())
    w = (e / e.sum()).astype(f32)

    # x transposed + edge-padded left by 23: col p+PADL = position p
    xT = np.ascontiguousarray(x.transpose(0, 2, 1))            # [B, C, L]
    xpad = np.concatenate(
        [np.repeat(xT[:, :, 0:1], PADL, axis=2), xT], axis=2)  # [B, C, L+23]

    # conv weights (fold mean 1/W into channel block 1)
    cw = conv_w.copy()
    cw[:, C:2 * C, :] /= W
    wr = np.empty((6, 128, 512), f32)
    for tap in range(3):
        for h in range(2):
            wr[tap * 2 + h] = cw[:, 128 * h:128 * (h + 1), tap].T

    M1 = mixer_w[:, :D]
    M2 = mixer_w[:, D:]
    F = M1 @ tproj_w
    F_g = F * g_c[None, :]
    c1 = F @ b_c + M1 @ tproj_b + mixer_b
    ftg = np.ascontiguousarray(F_g.T).reshape(4, 128, 512)
    m2t = np.ascontiguousarray(M2.T).reshape(4, 128, 512)

    pfg = (w[1] * (g_f[None, :] * _TABN + b_f[None, :])
           + (w[0] * b_c + w[3] * b_t + w[2] * b_l)[None, :]).astype(f32)

    grows = np.concatenate([w[0] * g_c, w[3] * g_t, w[2] * g_l])[None, :]

    # halo feature columns at positions 512c-1 and 512c+512 (mod L)
    def feat_col(g):
        win = xpad[:, :, g:g + W]                 # [B, C, 24] = pos g-23..g
        xv = xT[:, :, g]
        wsum = win.sum(-1)
        wmax = win.max(-1)
        wmin = win.min(-1)
        wstd = win.std(-1, ddof=1)
        lgs = [xv - xpad[:, :, g + PADL - lg_] for lg_ in LAGS]
        return [xv, wsum, wmax, wmin, wstd] + lgs  # 8 x [B, C]

    halo_cols = {}
    for c in range(NCORES):
        for g in ((LC * c - 1) % L, (LC * c + LC) % L):
            if g not in halo_cols:
                halo_cols[g] = feat_col(g)

    base = {
        "wr": _to_bf16(wr),
        "convb": _to_bf16(conv_b[None, :]),
        "ftg": _to_bf16(ftg),
        "m2t": _to_bf16(m2t),
        "c1": _to_bf16(c1[None, :]),
        "grows": grows.astype(f32),
        "ident": _to_bf16(np.eye(128, dtype=f32)),
        "ones1": _to_bf16(np.ones((1, 128), f32)),
        "ones1f": np.ones((1, 128), f32),
    }
    in_maps = []
    for c in range(NCORES):
        m = dict(base)
        xs = xpad[:, :, LC * c:LC * c + XSL]       # [B, C, 535]
        # partition layout p = ch*4 + bl
        m["xs"] = np.ascontiguousarray(
            xs.reshape(NBT, 4, C, XSL).transpose(0, 2, 1, 3)
            .reshape(NBT, 128, XSL))
        halo = np.empty((2, 128, 2 * B), f32)
        for s, g in enumerate(((LC * c - 1) % L, (LC * c + LC) % L)):
            fc = halo_cols[g]
            for f in range(8):
                h, r = f // 4, f % 4
                halo[h, 32 * r:32 * (r + 1), s::2] = fc[f].T
        m["halo"] = _to_bf16(halo)
        tp = tape_pos[LC * c:LC * (c + 1), :]      # [512, 512]
        m["tapetc"] = _to_bf16(np.ascontiguousarray(tp.T).reshape(4, 128, 512))
        lp = learned_pe[0, LC * c:LC * (c + 1)].reshape(NTL, 128, 512)
        pf = pfg[LC * c:LC * (c + 1)].reshape(NTL, 128, 512)
        m["pelpf"] = np.concatenate([lp, pf], axis=2).copy()
        in_maps.append(m)
    return in_maps


def kernel(**inputs):
    in_maps = _host_prep(inputs)
    nc = _get_nc()
    res = run_bass_kernel_spmd(nc, in_maps, core_ids=list(range(NCORES)))
    out = np.concatenate([r["out"] for r in res.results], axis=1)
    return out.astype(np.float32)



# revision 2
# speedup vs baseline: 1.0570x; 1.0570x over previous
"""Trainium2 Bass kernel for nn_DataEmbedding_ALLPE_Weighted (v3).

Sequence-parallel over L (4096/8 = 512 positions/core). The device program is
a tile-level software pipeline over 64 (batch, token-tile) units: conv runs
~2 pairs ahead of the mixer, LN smalls are batched per tile-pair, and the
rolling-stats phase A is interleaved into the pipeline.

Parameter-only terms are folded on the host: pe_lin = tape_pos @ M2.T + c1
(tape positional mixer term) and R0 = w1*LN(pe_fixed) + w2*LN(learned_pe)
+ bias combo are precomputed and DMA'd as bf16 constants, so the device only
runs: rolling stats -> conv -> LN-c -> mixer -> LN-t -> weighted epilogue.

When gains are uniform (spec: ones) the epilogue is two scalar_tensor_tensor
ops (Pool + DVE) with float immediates; general inputs fall back to
broadcast-gain tensor ops. Zero conv bias (spec) drops the bias matmul.
Output tensor is bf16 on device, converted to f32 on host.
"""

import numpy as np

import concourse.bass as bass
import concourse.mybir as mybir
import concourse.tile as tile
from concourse import bacc
from concourse.bass_utils import run_bass_kernel_spmd

F32 = mybir.dt.float32
BF16 = mybir.dt.bfloat16
AL = mybir.AluOpType
AF = mybir.ActivationFunctionType

B, L, C, D = 16, 4096, 32, 512
W = 24
LAGS = [3, 5, 7]
EPS = 1e-5
NCORES = 8
LC = L // NCORES           # 512 positions per core
NBT = 4                    # batch tiles of 4 batches
NTL = LC // 128            # 4 token tiles per batch
PADL = W - 1               # 23
XSL = LC + PADL            # 535
CB = LC + 2                # 514 comb cols per batch
NK = B * NTL               # 64 pipeline tiles
NP = NK // 2               # 32 tile pairs
LP = 2                     # pair lookahead of conv ahead of mixer


def _build_bass(uniform, w0s, w3s, zero_bias):
    nc = bacc.Bacc("TRN2", target_bir_lowering=False, debug=False,
                   num_devices=NCORES)

    def din(name, shape, dt=F32):
        return nc.dram_tensor(name, shape, dt, kind="ExternalInput").ap()

    t_xs = din("xs", [NBT, 128, XSL])
    t_halo = din("halo", [4, 128, 16], BF16)  # idx = h*2 + side
    t_wr = din("wr", [6, 128, 512], BF16)        # idx = tap*2 + half
    t_ftg = din("ftg", [4, 128, 512], BF16)
    t_pelin = din("pelin", [NTL, 128, 512], BF16)
    t_ident = din("ident", [128, 128], BF16)
    t_r0 = din("r0", [NTL, 128, 512], BF16)
    t_convb = din("convb", [1, 512], BF16) if not zero_bias else None
    t_ones1 = din("ones1", [1, 128], BF16) if not zero_bias else None
    if not uniform:
        t_grows = din("grows", [1, 2 * 512])     # w0*g_c | w3*g_t
        t_ones1f = din("ones1f", [1, 128])
    else:
        t_grows = t_ones1f = None
    t_out = nc.dram_tensor("out", [B, LC, D], BF16, kind="ExternalOutput").ap()

    with tile.TileContext(nc) as tc:
        _body(tc, nc, uniform, w0s, w3s, zero_bias,
              t_xs, t_halo, t_wr, t_ftg, t_pelin, t_r0, t_ident,
              t_convb, t_ones1, t_grows, t_ones1f, t_out)
    nc.compile()
    return nc


def _body(tc, nc, uniform, w0s, w3s, zero_bias,
          t_xs, t_halo, t_wr, t_ftg, t_pelin, t_r0, t_ident,
          t_convb, t_ones1, t_grows, t_ones1f, t_out):
    v = nc.vector
    gp = nc.gpsimd
    sc = nc.scalar
    pe = nc.tensor
    sy = nc.sync

    cpool = tc.alloc_tile_pool(name="consts", bufs=1)
    comb_s = cpool.tile([128, NBT * 2 * 4 * CB], BF16, name="comb")

    def comb_v(t, h):
        off = (t * 2 + h) * 4 * CB
        return comb_s[:, off:off + 4 * CB]

    def emit_halo(t):
        # halo columns (2 per batch) into comb for batch-tile t: 4 DMAs
        for h in range(2):
            dstv = comb_v(t, h).rearrange("p (b k) -> p b k", k=CB)
            for side, col in ((0, 0), (1, CB - 1)):
                sc.dma_start(dstv[:, :, col:col + 1],
                             t_halo[h * 2 + side, :, 4 * t:4 * (t + 1)]
                             .rearrange("p (b o) -> p b o", o=1))

    eps_s = cpool.tile([128, 1], F32)
    gp.memset(eps_s[:], EPS)
    wc2 = (1.0 / (w0s * w0s)) if uniform else 1.0
    wt2 = (1.0 / (w3s * w3s)) if uniform else 1.0
    epsc_s = cpool.tile([128, 1], F32)
    gp.memset(epsc_s[:], EPS * wc2)
    epst_s = cpool.tile([128, 1], F32)
    gp.memset(epst_s[:], EPS * wt2)
    zscan_s = cpool.tile([128, XSL], F32)
    gp.memset(zscan_s[:], 0.0)

    wr_s = cpool.tile([128, 6 * 512], BF16)
    sy.dma_start(wr_s[:].rearrange("p (k n) -> p k n", k=6),
                 t_wr.rearrange("k p n -> p k n"))
    ftg_s = cpool.tile([128, 4 * 512], BF16)
    sy.dma_start(ftg_s[:].rearrange("p (k n) -> p k n", k=4),
                 t_ftg.rearrange("k p n -> p k n"))
    pelin_sb = cpool.tile([128, NTL * 512], BF16)
    sy.dma_start(pelin_sb[:].rearrange("p (k n) -> p k n", k=NTL),
                 t_pelin.rearrange("k p n -> p k n"))
    r0_sb = cpool.tile([128, NTL * 512], BF16)
    sy.dma_start(r0_sb[:].rearrange("p (k n) -> p k n", k=NTL),
                 t_r0.rearrange("k p n -> p k n"))
    ident_s = cpool.tile([128, 128], BF16)
    sy.dma_start(ident_s[:], t_ident)
    if not zero_bias:
        convb_s = cpool.tile([1, 512], BF16)
        sy.dma_start(convb_s[:], t_convb)
        ones1_s = cpool.tile([1, 128], BF16)
        sy.dma_start(ones1_s[:], t_ones1)
    psA = tc.alloc_tile_pool(name="psA", bufs=6, space="PSUM")
    psB = tc.alloc_tile_pool(name="psB", bufs=2, space="PSUM")
    pa = tc.alloc_tile_pool(name="pa", bufs=2)
    phb = tc.alloc_tile_pool(name="phb", bufs=12)   # tsb: mixer evac
    pmc = tc.alloc_tile_pool(name="pmc", bufs=8)    # hc, hcT
    po1 = tc.alloc_tile_pool(name="po1", bufs=17)   # o1 (long-lived)
    pmt = tc.alloc_tile_pool(name="pmt", bufs=4)    # ht, osb
    st = tc.alloc_tile_pool(name="st", bufs=12)

    if not uniform:
        bc_s = cpool.tile([128, 2 * 512], F32)   # g0bc | g3bc
        bcb_s = cpool.tile([128, 2 * 512], BF16)
        ones1f_s = cpool.tile([1, 128], F32)
        sy.dma_start(ones1f_s[:], t_ones1f)
        grows_s = cpool.tile([1, 2 * 512], F32)
        sy.dma_start(grows_s[:], t_grows)
        for i in range(2):
            ps = psA.tile([128, 512], F32, tag="ph")
            pe.matmul(ps[:], ones1f_s[:], grows_s[:, 512 * i:512 * (i + 1)],
                      start=True, stop=True)
            sc.copy(bc_s[:, 512 * i:512 * (i + 1)], ps[:])
        sc.copy(bcb_s[:], bc_s[:])

    # ---- Phase A: rolling stats for one batch-tile, split into chunks ----
    def phase_a_chunks(t, early=False):
        state = {}
        ttv = v if early else gp

        def c_load():
            xs = pa.tile([128, XSL], F32, tag="xs")
            sy.dma_start(xs[:], t_xs[t])
            xsb = pa.tile([128, XSL], BF16, tag="xsb")
            v.tensor_copy(xsb[:], xs[:])
            xsq = pa.tile([128, XSL], F32, tag="xsq")
            sc.activation(xsq[:], xs[:], AF.Square)
            state.update(xs=xs, xsb=xsb, xsq=xsq)

        def c_scan():
            xs, xsq = state["xs"], state["xsq"]
            cs = pa.tile([128, XSL + 1], F32, tag="cs")
            gp.memset(cs[:, 0:1], 0.0)
            v.tensor_tensor_scan(cs[:, 1:XSL + 1], zscan_s[:], xs[:], 0.0,
                                 op0=AL.add, op1=AL.add)
            cs2 = pa.tile([128, XSL + 1], F32, tag="cs2")
            gp.memset(cs2[:, 0:1], 0.0)
            v.tensor_tensor_scan(cs2[:, 1:XSL + 1], zscan_s[:], xsq[:], 0.0,
                                 op0=AL.add, op1=AL.add)
            wsum = pa.tile([128, LC], BF16, tag="wsum")
            v.tensor_sub(wsum[:], cs[:, W:XSL + 1], cs[:, 0:LC])
            ssqw = pa.tile([128, LC], F32, tag="ssqw")
            gp.tensor_sub(ssqw[:], cs2[:, W:XSL + 1], cs2[:, 0:LC])
            wsq = pa.tile([128, LC], F32, tag="wsq")
            v.tensor_mul(wsq[:], wsum[:], wsum[:])
            var23 = pa.tile([128, LC], F32, tag="var23")
            v.scalar_tensor_tensor(var23[:], wsq[:], -1.0 / W, ssqw[:],
                                   op0=AL.mult, op1=AL.add)
            v.tensor_scalar_max(var23[:], var23[:], 0.0)
            stdt = pa.tile([128, LC], BF16, tag="stdt")
            sc.activation(stdt[:], var23[:], AF.Sqrt, scale=1.0 / (W - 1))
            state.update(wsum=wsum, stdt=stdt)

        def c_maxmin():
            xsb = state["xsb"]
            outs = {}
            for name, op, eng in (("mx", AL.max, v), ("mn", AL.min, v)):
                m2c = pa.tile([128, XSL - 1], BF16, tag=name + "2")
                eng.tensor_tensor(m2c[:], xsb[:, 0:XSL - 1], xsb[:, 1:XSL],
                                  op=op)
                m4 = pa.tile([128, XSL - 3], BF16, tag=name + "4")
                eng.tensor_tensor(m4[:], m2c[:, 0:XSL - 3], m2c[:, 2:XSL - 1],
                                  op=op)
                m8 = pa.tile([128, XSL - 7], BF16, tag=name + "8")
                eng.tensor_tensor(m8[:], m4[:, 0:XSL - 7], m4[:, 4:XSL - 3],
                                  op=op)
                m16 = pa.tile([128, XSL - 15], BF16, tag=name + "16")
                eng.tensor_tensor(m16[:], m8[:, 0:XSL - 15], m8[:, 8:XSL - 7],
                                  op=op)
                mo = pa.tile([128, LC], BF16, tag=name + "o")
                eng.tensor_tensor(mo[:], m16[:, 8:LC + 8], m8[:, 0:LC], op=op)
                outs[name] = mo
            state.update(outs)

        def c_lags():
            xsb = state["xsb"]
            lags = []
            for lg_ in LAGS:
                lg = pa.tile([128, LC], BF16, tag=f"lag{lg_}")
                ttv.tensor_sub(lg[:], xsb[:, PADL:XSL],
                               xsb[:, PADL - lg_:XSL - lg_])
                lags.append(lg)
            state["lags"] = lags

        def c_store():
            emit_halo(t)
            feats = [[state["xsb"][:, PADL:XSL], state["wsum"][:],
                      state["mx"][:], state["mn"][:]],
                     [state["stdt"][:]] + [lg[:] for lg in state["lags"]]]
            for h in range(2):
                for r, ft in enumerate(feats[h]):
                    dst = comb_v(t, h)[32 * r:32 * (r + 1), :].rearrange(
                        "p (b k) -> p b k", k=CB)
                    sy.dma_start(dst[:, :, 1:1 + LC], ft)

        return [c_load, c_scan, c_maxmin, c_lags, c_store]

    # ---- pipeline stages ----
    # LN smalls are batched per group of G tiles: one DVE->Act->DVE round
    # trip per group. Both conv and mixer PSUM results are evacuated to
    # fp16 SBUF immediately, so PSUM rings stay tiny and stage lags can be
    # deep without bank pressure.
    GC = 4      # LN-c smalls group (conv PSUM held until hc)
    GT = 8      # LN-t smalls group (mixer evacuated to SBUF)
    phs = {}    # k -> conv PSUM tile
    ppts = {}   # k -> mixer PSUM tile
    tsbs = {}   # k -> evacuated mixer SBUF tile (fp16)
    bns = {}    # (tag, k) -> bn6 tile
    smc = {}    # group -> (al, be) for LN-c
    smt = {}    # group -> (al, be) for LN-t
    hcs = {}    # k -> hc
    hcTs = {}   # k -> transposed hc
    o1s = {}    # k -> first epilogue term
    osbs = {}   # k -> output tile
    F16 = mybir.dt.float16

    def s_conv(k):
        b = k // NTL
        il = k % NTL
        t, bl = b // 4, b % 4
        col = CB * bl + 128 * il
        ph = psA.tile([128, 512], F32, tag="ph")
        kk = 0
        for tap in range(3):
            for h in range(2):
                pe.matmul(ph[:], comb_v(t, h)[:, col + tap:col + tap + 128],
                          wr_s[:, 512 * (tap * 2 + h):
                               512 * (tap * 2 + h) + 512],
                          start=(kk == 0),
                          stop=zero_bias and (kk == 5))
                kk += 1
        if not zero_bias:
            pe.matmul(ph[:], ones1_s[:], convb_s[:], start=False, stop=True)
        phs[k] = ph

    def s_evac_t(k):
        ppt = ppts.pop(k)
        tsb = phb.tile([128, 512], BF16, tag="tsb")
        sc.copy(tsb[:], ppt[:])
        tsbs[k] = tsb

    def s_bnc(k):
        bn = st.tile([128, 6], F32, tag="bnc")
        v.bn_stats(bn[:], phs[k][:])
        bns[("c", k)] = bn

    def s_bnt(k):
        bn = st.tile([128, 6], F32, tag="bnt")
        v.bn_stats(bn[:], tsbs[k][:])
        bns[("t", k)] = bn

    def s_smalls_grp(g, tagp, out_map, gsz):
        # layout: cols 0:gsz = means (per tile), cols gsz:2*gsz = vars
        # se = sqrt((var+eps)/w^2) so al = w/sigma folds the epilogue
        # weight into the LN scale (w=1 on the general path)
        w2, ept = (wc2, epsc_s) if tagp == "c" else (wt2, epst_s)
        mvg = st.tile([128, 2 * gsz], F32, tag="mv" + tagp)
        mvv = mvg[:].rearrange("p (a b) -> p a b", a=2)
        for i in range(gsz):
            bn = bns.pop((tagp, gsz * g + i))
            v.bn_aggr(mvv[:, :, i:i + 1], bn[:])
        se = st.tile([128, gsz], F32, tag="se" + tagp)
        sc.activation(se[:], mvg[:, gsz:2 * gsz], AF.Sqrt,
                      bias=ept[:], scale=w2)
        al = st.tile([128, gsz], F32, tag="al" + tagp)
        v.reciprocal(al[:], se[:])
        be = st.tile([128, gsz], F32, tag="be" + tagp)
        v.scalar_tensor_tensor(be[:], mvg[:, 0:gsz], -1.0, al[:],
                               op0=AL.mult, op1=AL.mult)
        out_map[g] = (al, be)

    def s_hc(k):
        al, be = smc[k // GC]
        i = k % GC
        ph = phs.pop(k)
        hc = pmc.tile([128, 512], BF16, tag="hc")
        sc.activation(hc[:], ph[:], AF.Identity,
                      bias=be[:, i:i + 1], scale=al[:, i:i + 1])
        hcs[k] = hc

    def s_tr(k):
        # issued one iter after s_hc: hc is complete, no queue blocking
        hcT = pmc.tile([128, 512], BF16, tag="hcT")
        sy.dma_start_transpose(
            hcT[:].rearrange("p (j n) -> p j n", j=4), hcs[k][:])
        hcTs[k] = hcT

    def s_o1(k):
        il = k % NTL
        hc = hcs.pop(k)
        o1 = po1.tile([128, 512], BF16, tag="o1")
        if uniform:
            gp.tensor_tensor(o1[:], hc[:],
                             r0_sb[:, 512 * il:512 * (il + 1)], op=AL.add)
        else:
            o1f = pmc.tile([128, 512], BF16, tag="o1f")
            gp.tensor_tensor(o1f[:], hc[:], bcb_s[:, 0:512], op=AL.mult)
            gp.tensor_tensor(o1[:], o1f[:],
                             r0_sb[:, 512 * il:512 * (il + 1)], op=AL.add)
        o1s[k] = o1

    def s_mix(k):
        il = k % NTL
        hcT = hcTs.pop(k)
        ppt = psB.tile([128, 512], F32, tag="ppt")
        for j in range(4):
            pe.matmul(ppt[:], hcT[:, 128 * j:128 * (j + 1)],
                      ftg_s[:, 512 * j:512 * (j + 1)],
                      start=(j == 0), stop=False)
        pe.matmul(ppt[:], ident_s[:],
                  pelin_sb[:, 512 * il:512 * (il + 1)],
                  start=False, stop=True)
        ppts[k] = ppt

    def s_ht_epi(k):
        al, be = smt[k // GT]
        i = k % GT
        tsb = tsbs.pop(k)
        ht = pmt.tile([128, 512], BF16, tag="ht")
        sc.activation(ht[:], tsb[:], AF.Identity,
                      bias=be[:, i:i + 1], scale=al[:, i:i + 1])
        o1 = o1s.pop(k)
        osb = pmt.tile([128, 512], BF16, tag="osb")
        if uniform:
            v.tensor_tensor(osb[:], ht[:], o1[:], op=AL.add)
        else:
            m1 = pmt.tile([128, 512], BF16, tag="m1")
            v.tensor_tensor(m1[:], ht[:], bcb_s[:, 512:1024], op=AL.mult)
            v.tensor_tensor(osb[:], m1[:], o1[:], op=AL.add)
        osbs[k] = osb

    def s_out(k):
        b = k // NTL
        il = k % NTL
        osb = osbs.pop(k)
        sy.dma_start(t_out[b, 128 * il:128 * (il + 1), :], osb[:])

    # ---- emission: staged pipeline with deep offsets ----
    for ch in phase_a_chunks(0, early=True):
        ch()
    for ch in phase_a_chunks(1, early=True):
        ch()

    a_sched = {}
    for t in (2, 3):
        for j, ch in enumerate(phase_a_chunks(t)):
            a_sched.setdefault(16 * t - 22 + 3 * j, []).append(ch)

    D_CONV, D_BN, D_HC, D_TR = 8, 7, 4, 3
    for k in range(-D_CONV, NK + 11):
        if 0 <= k + D_CONV < NK:
            s_conv(k + D_CONV)
        if 0 <= k + D_BN < NK:
            s_bnc(k + D_BN)
            if (k + D_BN) % GC == GC - 1:
                s_smalls_grp((k + D_BN) // GC, "c", smc, GC)
        if 0 <= k + D_HC < NK:
            s_hc(k + D_HC)
        if 0 <= k + D_TR < NK:
            s_tr(k + D_TR)
            s_o1(k + D_TR)
        if 0 <= k < NK:
            s_mix(k)
            s_evac_t(k)
        if 0 <= k - 1 < NK:
            s_bnt(k - 1)
            if (k - 1) % GT == GT - 1:
                s_smalls_grp((k - 1) // GT, "t", smt, GT)
        if 0 <= k - 9 < NK:
            s_ht_epi(k - 9)
        if 0 <= k - 10 < NK:
            s_out(k - 10)
        for ch in a_sched.get(k, ()):
            ch()

    for p in (st, pmt, po1, pmc, phb, pa, psB, psA, cpool):
        p.release()


_NC_CACHE = {}


def _get_nc(key=(True, 0.25, 0.25, True)):
    if key not in _NC_CACHE:
        _NC_CACHE[key] = _build_bass(*key)
    return _NC_CACHE[key]


def _to_bf16(a):
    import ml_dtypes
    return np.asarray(a, np.float32).astype(ml_dtypes.bfloat16)


def _fixed_pe_fold():
    """Normalized fixed sinusoidal PE table [L, D] (input-independent)."""
    pos = np.arange(L, dtype=np.float64)
    div = np.exp(np.arange(0, D, 2, dtype=np.float64) * (-np.log(10000.0) / D))
    ang = pos[:, None] * div[None, :]
    tab = np.zeros((L, D), np.float64)
    tab[:, 0::2] = np.sin(ang)
    tab[:, 1::2] = np.cos(ang)
    tab = tab.astype(np.float32)
    return _ln_rows(tab)


def _ln_rows(a):
    m = a.mean(-1, keepdims=True)
    vv = ((a - m) ** 2).mean(-1, keepdims=True)
    return ((a - m) / np.sqrt(vv + EPS)).astype(np.float32)


_TABN = _fixed_pe_fold()


def _host_prep(inputs):
    f32 = np.float32
    x = np.asarray(inputs["x"], f32)
    conv_w = np.asarray(inputs["conv_w"], f32)
    conv_b = np.asarray(inputs["conv_b"], f32)
    learned_pe = np.asarray(inputs["learned_pe"], f32)
    tape_pos = np.asarray(inputs["tape_pos"], f32)
    tproj_w = np.asarray(inputs["tproj_w"], f32)
    tproj_b = np.asarray(inputs["tproj_b"], f32)
    mixer_w = np.asarray(inputs["mixer_w"], f32)
    mixer_b = np.asarray(inputs["mixer_b"], f32)
    g_c, b_c = np.asarray(inputs["g_c"], f32), np.asarray(inputs["b_c"], f32)
    g_f, b_f = np.asarray(inputs["g_f"], f32), np.asarray(inputs["b_f"], f32)
    g_l, b_l = np.asarray(inputs["g_l"], f32), np.asarray(inputs["b_l"], f32)
    g_t, b_t = np.asarray(inputs["g_t"], f32), np.asarray(inputs["b_t"], f32)
    wp = np.asarray(inputs["weight_params"], f32)

    e = np.exp(wp - wp.max())
    w = (e / e.sum()).astype(f32)

    # x transposed + edge-padded left by 23: col p+PADL = position p
    xT = np.ascontiguousarray(x.transpose(0, 2, 1))            # [B, C, L]
    xpad = np.concatenate(
        [np.repeat(xT[:, :, 0:1], PADL, axis=2), xT], axis=2)  # [B, C, L+23]

    # conv weights (fold mean 1/W into channel block 1)
    cw = conv_w.copy()
    cw[:, C:2 * C, :] /= W
    wr = np.empty((6, 128, 512), f32)
    for tap in range(3):
        for h in range(2):
            wr[tap * 2 + h] = cw[:, 128 * h:128 * (h + 1), tap].T

    M1 = mixer_w[:, :D]
    M2 = mixer_w[:, D:]
    F = M1 @ tproj_w
    F_g = F * g_c[None, :]
    c1 = F @ b_c + M1 @ tproj_b + mixer_b
    ftg_m = F_g.T

    # host-folded positional terms (parameter-only)
    pe_lin = tape_pos[:L] @ M2.T + c1[None, :]                 # [L, D]
    r0 = (w[1] * (g_f[None, :] * _TABN + b_f[None, :])
          + w[2] * (g_l[None, :] * _ln_rows(learned_pe[0, :L]))
          + (w[0] * b_c + w[3] * b_t + w[2] * b_l)[None, :]).astype(f32)

    zero_bias = not conv_b.any()
    uniform = (np.ptp(g_c) == 0 and np.ptp(g_t) == 0
               and g_c[0] > 0 and g_t[0] > 0)
    w0s = float(w[0] * g_c[0]) if uniform else 0.25
    w3s = float(w[3] * g_t[0]) if uniform else 0.25
    key = (uniform, w0s, w3s, zero_bias)
    if uniform:
        # hc/ht carry the w0*g_c / w3*g_t factors via the LN scale;
        # compensate the mixer weights which consume hc
        ftg_m = ftg_m / w0s
    ftg = np.ascontiguousarray(ftg_m).reshape(4, 128, 512)

    # halo feature columns at positions 512c-1 and 512c+512 (mod L)
    def feat_col(g):
        win = xpad[:, :, g:g + W]                 # [B, C, 24] = pos g-23..g
        xv = xT[:, :, g]
        wsum = win.sum(-1)
        wmax = win.max(-1)
        wmin = win.min(-1)
        wstd = win.std(-1, ddof=1)
        lgs = [xv - xpad[:, :, g + PADL - lg_] for lg_ in LAGS]
        return [xv, wsum, wmax, wmin, wstd] + lgs  # 8 x [B, C]

    halo_cols = {}
    for c in range(NCORES):
        for g in ((LC * c - 1) % L, (LC * c + LC) % L):
            if g not in halo_cols:
                halo_cols[g] = feat_col(g)

    base = {
        "wr": _to_bf16(wr),
        "ftg": _to_bf16(ftg),
        "ident": _to_bf16(np.eye(128, dtype=f32)),
    }
    if not zero_bias:
        base["convb"] = _to_bf16(conv_b[None, :])
        base["ones1"] = _to_bf16(np.ones((1, 128), f32))
    if not uniform:
        grows = np.concatenate([w[0] * g_c, w[3] * g_t])[None, :]
        base["grows"] = grows.astype(f32)
        base["ones1f"] = np.ones((1, 128), f32)
    in_maps = []
    for c in range(NCORES):
        m = dict(base)
        xs = xpad[:, :, LC * c:LC * c + XSL]       # [B, C, 535]
        # partition layout p = ch*4 + bl
        m["xs"] = np.ascontiguousarray(
            xs.reshape(NBT, 4, C, XSL).transpose(0, 2, 1, 3)
            .reshape(NBT, 128, XSL))
        halo = np.empty((4, 128, 16), f32)
        for s_, g in enumerate(((LC * c - 1) % L, (LC * c + LC) % L)):
            fc = halo_cols[g]
            for f in range(8):
                h, r = f // 4, f % 4
                # col index = t*4 + b for batch index t*4+b
                halo[h * 2 + s_, 32 * r:32 * (r + 1), :] = fc[f].T
        m["halo"] = _to_bf16(halo)
        m["pelin"] = _to_bf16(
            pe_lin[LC * c:LC * (c + 1)].reshape(NTL, 128, 512))
        m["r0"] = _to_bf16(r0[LC * c:LC * (c + 1)].reshape(NTL, 128, 512))
        in_maps.append(m)
    return in_maps, key


def kernel(**inputs):
    in_maps, key = _host_prep(inputs)
    nc = _get_nc(key)
    res = run_bass_kernel_spmd(nc, in_maps, core_ids=list(range(NCORES)))
    out = np.concatenate([r["out"] for r in res.results], axis=1)
    return out.astype(np.float32)
